# revision 10
# baseline (speedup 1.0000x reference)
import numpy as np
import concourse.bass as bass
import concourse.tile as tile
from concourse import mybir
from concourse.bass_utils import run_bass_kernel_spmd
from concourse.masks import make_identity

P = 128
S = 2048
D = 512
U = 1024
NS = S // P      # 16 s-tiles
ND = D // P      # 4 d-blocks
NEG = -60000.0
EPS = 1e-6


def _patched_drain_and_barrier(self, tick_clock, wait_clock):
    nc = self.nc
    probe = nc.sync.nop(nofuse=True, hint="drain_waits_probe")
    wait_clock.add_sem_waits(probe.ins, tile.ScopedClock({None: tick_clock.global_clock}))
    si = probe.ins.sync_info
    waits = list(si.on_wait) if si is not None else []
    assert self.sems is not None
    handles = {h.name: h for h in self.sems.allocated().values()}
    if len(waits) > 1:
        import bass_rust
        probe.ins.sync_info = bass_rust.SyncInfo(on_wait=waits[:1], on_update=[])
        for w in waits[1:]:
            h = handles.get(w.ant_name)
            assert h is not None, (w.ant_name, list(handles))
            nc.sync.wait_ge(h, w.wait_value)
    nc.sync.drain()
    nc.all_engine_barrier()
    popped = nc._tile_sem_poison_stack.pop()
    assert popped is self._sem_poison
    nc.clear_and_free_semaphores(list(self.sems.allocated().values()))
    nc.all_engine_barrier()


tile.TileContext._drain_and_barrier = _patched_drain_and_barrier

# The walrus backend in this toolchain rejects instructions carrying more
# than one semaphore wait ("Too many sync wait commands"). Split excess
# waits onto single-wait NoOp carriers on the same engine, which execute
# in order ahead of the real instruction.
_MAXW = 1
_orig_lower_ordered = tile.TileContext._lower_ordered_insts


def _patched_lower_ordered(self, ordered):
    nc = self.nc
    for insts in ordered.values():
        out = []
        for inst in insts:
            si = getattr(inst, "sync_info", None)
            eng = getattr(inst, "engine", None)
            if (si is not None and si.on_wait and len(si.on_wait) > _MAXW
                    and eng is not None
                    and not type(inst).__name__.startswith("BassTile")):
                waits = list(si.on_wait)
                for w in waits[:-_MAXW]:
                    out.append(mybir.InstNoOp(
                        name=nc.get_next_instruction_name(),
                        engine=eng,
                        ins=[],
                        outs=[],
                        bass_nofuse=True,
                        sync_info=mybir.SyncInfo(on_wait=[w], on_update=[]),
                    ))
                inst.sync_info = mybir.SyncInfo(
                    on_wait=waits[-_MAXW:], on_update=list(si.on_update))
            out.append(inst)
        insts[:] = out
    return _orig_lower_ordered(self, ordered)


tile.TileContext._lower_ordered_insts = _patched_lower_ordered

f32 = mybir.dt.float32
f16 = mybir.dt.float16
bf16 = mybir.dt.bfloat16


def _build():
    nc = bass.Bass()
    # Per-core inputs (1 batch element, 2 heads):
    #   x   [S, D]    activations
    #   ub  [P, 2*ND] per-head score key-side bias (beta @ Wq @ (g*Wk)^T),
    #                 column h*ND+j holds entries d = j*128 + p
    #   a   [2D, D]   A_h = (g*Wq_h)(g*Wk_h)^T stacked over the 2 heads, f16
    #   n   [2D, D]   N_h = (g*Wv_h) Wout_h stacked, f16
    # scores = (z @ A + u) @ z^T ; out = sum_h probs_h @ (z @ N_h) / Z_h
    x_ext = nc.declare_dram_parameter("x", [S, D], f32, isOutput=False)
    ub_ext = nc.declare_dram_parameter("ub", [P, 2 * ND], f32, isOutput=False)
    a_ext = nc.declare_dram_parameter("a", [2 * D, D], f16, isOutput=False)
    n_ext = nc.declare_dram_parameter("n", [2 * D, D], f16, isOutput=False)
    out_ext = nc.declare_dram_parameter("out", [S, D], f32, isOutput=True)

    with tile.TileContext(nc) as tc:
        with tc.tile_pool(name="const", bufs=1) as cp, \
             tc.tile_pool(name="znt", bufs=1) as xp, \
             tc.tile_pool(name="wp", bufs=1) as wp, \
             tc.tile_pool(name="qkv", bufs=1) as qp, \
             tc.tile_pool(name="ln", bufs=2) as lp, \
             tc.tile_pool(name="xd", bufs=16) as xdp, \
             tc.tile_pool(name="att", bufs=2) as ap_, \
             tc.tile_pool(name="st", bufs=2) as sp, \
             tc.tile_pool(name="oacc", bufs=1) as op, \
             tc.tile_pool(name="outp", bufs=2) as up, \
             tc.tile_pool(name="mm", bufs=2, space="PSUM") as mmp, \
             tc.tile_pool(name="sc", bufs=2, space="PSUM") as scp, \
             tc.tile_pool(name="pv", bufs=1, space="PSUM") as pvp, \
             tc.tile_pool(name="tr", bufs=2, space="PSUM") as trp, \
             tc.tile_pool(name="trl", bufs=1, space="PSUM") as trlp:

            dmaq = [nc.sync, nc.scalar, nc.gpsimd]

            # ---- stage all DMAs up front: x tiles first, weights behind ----
            xts = []
            for i in range(NS):
                xt = xdp.tile([P, D], f32, tag="x", name=f"xt{i}")
                xts.append(xt)

            def load_w(w_ext_, h, tagc, engines):
                wt = [wp.tile([P, D], f16, tag=f"{tagc}{h}_{k}", name=f"{tagc}{h}_{k}")
                      for k in range(ND)]
                for k in range(ND):
                    engines[k].dma_start(
                        out=wt[k][:],
                        in_=w_ext_[h * D + k * P: h * D + (k + 1) * P, :])
                return wt

            def dma_x(i):
                dmaq[i % 3].dma_start(out=xts[i][:], in_=x_ext[i * P:(i + 1) * P, :])

            # interleave weight loads between early x tiles: everything funnels
            # into few hw queues, so issue order is arrival order
            for i in range(3):
                dma_x(i)
            ubt = cp.tile([P, 2 * ND], f32, tag="ubt")
            nc.scalar.dma_start(out=ubt[:], in_=ub_ext[:, :])
            at0 = load_w(a_ext, 0, "a", [nc.sync, nc.scalar, nc.sync, nc.scalar])
            for i in range(3, 6):
                dma_x(i)
            nt0 = load_w(n_ext, 0, "n", [nc.sync, nc.scalar, nc.sync, nc.scalar])
            at1 = load_w(a_ext, 1, "a", [nc.sync, nc.gpsimd, nc.sync, nc.gpsimd])
            for i in range(6, 10):
                dma_x(i)
            nt1 = load_w(n_ext, 1, "n", [nc.scalar, nc.gpsimd, nc.scalar, nc.gpsimd])
            for i in range(10, NS):
                dma_x(i)

            ident = cp.tile([P, P], f16, tag="ident")
            make_identity(nc, ident[:])
            identb = cp.tile([P, P], bf16, tag="identb")
            make_identity(nc, identb[:])
            eps = cp.tile([P, 1], f32, tag="eps")
            nc.vector.memset(eps[:], EPS)
            mask = cp.tile([P, 4 * D], f16, tag="mask")
            nc.gpsimd.memset(mask[:], 0.0)
            for m in range(4):
                # keep 0 where t <= m*128 + r, else NEG
                nc.gpsimd.affine_select(
                    out=mask[:, m * D:(m + 1) * D],
                    in_=mask[:, m * D:(m + 1) * D],
                    compare_op=mybir.AluOpType.is_ge,
                    fill=NEG,
                    base=m * P,
                    pattern=[[-1, D]],
                    channel_multiplier=1,
                )

            zT = [xp.tile([P, S], f16, tag=f"zt{j}", name=f"zt{j}") for j in range(ND)]
            oacc = [op.tile([P, D], bf16, tag=f"oacc{i}", name=f"oacc{i}") for i in range(NS)]
            qmT = [[qp.tile([P, S], f16, tag=f"qmt{h}_{j}", name=f"qmt{h}_{j}")
                    for j in range(ND)] for h in range(2)]
            vm = [[qp.tile([P, D], bf16, tag=f"vm{h}_{t}", name=f"vm{h}_{t}")
                   for t in range(NS)] for h in range(2)]

            def emit_ln_tile(i):
                xt = xts[i]
                stats = lp.tile([P, 6], f32, tag="bs", name="bs")
                nc.vector.bn_stats(out=stats[:], in_=xt[:])
                mv = lp.tile([P, 2], f32, tag="mv", name="mv")
                nc.vector.bn_aggr(out=mv[:], in_=stats[:])
                sd = lp.tile([P, 1], f32, tag="sd", name="sd")
                nc.scalar.activation(out=sd[:], in_=mv[:, 1:2],
                                     func=mybir.ActivationFunctionType.Sqrt,
                                     bias=eps[:], scale=1.0, alpha=0.0)
                nc.vector.reciprocal(out=sd[:], in_=sd[:])
                xh = lp.tile([P, D], f16, tag="xh", name="xh")
                norm_eng = nc.gpsimd if i % 2 else nc.vector
                norm_eng.tensor_scalar(out=xh[:], in0=xt[:],
                                       scalar1=mv[:, 0:1], scalar2=sd[:],
                                       op0=mybir.AluOpType.subtract,
                                       op1=mybir.AluOpType.mult)
                for j in range(ND):
                    tp = trlp.tile([P, P], f16, tag="tr", name="tp")
                    nc.tensor.transpose(tp[:], xh[:, j * P:(j + 1) * P], ident[:])
                    nc.any.tensor_copy(out=zT[j][:, i * P:(i + 1) * P], in_=tp[:])

            def emit_qm(h, at, g):
                # qmT[h][j][:, g*512:(g+1)*512] = A_h^T z^T + u  (d-tile j, s-chunk g)
                for j in range(ND):
                    mm = mmp.tile([P, D], f32, tag="mm", name="mm")
                    for k in range(ND):
                        nc.tensor.matmul(mm[:],
                                         at[k][:, j * P:(j + 1) * P],
                                         zT[k][:, g * D:(g + 1) * D],
                                         start=(k == 0), stop=(k == ND - 1))
                    nc.any.tensor_scalar_add(out=qmT[h][j][:, g * D:(g + 1) * D],
                                             in0=mm[:],
                                             scalar1=ubt[:, h * ND + j:h * ND + j + 1])

            def emit_vm(h, nt, t):
                # vm[h][t] = z N_h   (t-tile of rows)
                mm = mmp.tile([P, D], f32, tag="mm", name="mm")
                for k in range(ND):
                    nc.tensor.matmul(mm[:],
                                     zT[k][:, t * P:(t + 1) * P],
                                     nt[k][:, :],
                                     start=(k == 0), stop=(k == ND - 1))
                nc.any.tensor_copy(out=vm[h][t][:], in_=mm[:])

            def emit_scores(h, i):
                # scores row-tile i vs keys 0..(i+1)*128; exp without max-sub
                nch = i // 4 + 1
                Pt = ap_.tile([P, S], bf16, tag="P", name="Pt")
                rsum = sp.tile([P, 4], f32, tag="rsum", name="rsum")
                for c in range(nch):
                    w = (i % 4 + 1) * P if c == i // 4 else D
                    sc = scp.tile([P, D], f32, tag="sc", name="sc")
                    for k in range(ND):
                        nc.tensor.matmul(sc[:, 0:w],
                                         qmT[h][k][:, i * P:(i + 1) * P],
                                         zT[k][:, c * D:c * D + w],
                                         start=(k == 0), stop=(k == ND - 1))
                    if c == i // 4:
                        m = i % 4
                        nc.vector.tensor_add(out=sc[:, 0:w], in0=sc[:, 0:w],
                                             in1=mask[:, m * D:m * D + w])
                    nc.scalar.activation(out=Pt[:, c * D:c * D + w], in_=sc[:, 0:w],
                                         func=mybir.ActivationFunctionType.Exp,
                                         scale=1.0,
                                         accum_out=rsum[:, c:c + 1])
                return Pt, rsum

            def emit_tail(h, i, Pt, rsum, final=False):
                nch = i // 4 + 1
                # 1/Z
                tot = sp.tile([P, 1], f32, tag="tot", name="tot")
                if nch > 1:
                    nc.vector.reduce_sum(out=tot[:], in_=rsum[:, 0:nch],
                                         axis=mybir.AxisListType.X)
                    nc.vector.reciprocal(out=tot[:], in_=tot[:])
                else:
                    nc.vector.reciprocal(out=tot[:], in_=rsum[:, 0:1])
                # transpose probs blocks 0..i, 4 blocks per PSUM bank
                pt = ap_.tile([P, S], bf16, tag="pt", name="pt")
                for gr in range((i + 4) // 4):
                    tpw = min(4, i + 1 - gr * 4)
                    tp = trp.tile([P, D], bf16, tag="trb", name="tpb")
                    for q in range(tpw):
                        tb = gr * 4 + q
                        nc.tensor.matmul(tp[:, q * P:(q + 1) * P],
                                         Pt[:, tb * P:(tb + 1) * P], identb[:],
                                         is_transpose=True, skip_group_check=True)
                    nc.any.tensor_copy(out=pt[:, gr * D:gr * D + tpw * P],
                                       in_=tp[:, 0:tpw * P])
                # probs @ vm
                pv = pvp.tile([P, D], f32, tag="pv", name="pv")
                for tb in range(i + 1):
                    nc.tensor.matmul(pv[:],
                                     pt[:, tb * P:(tb + 1) * P],
                                     vm[h][tb][:],
                                     start=(tb == 0), stop=(tb == i))
                if h == 0:
                    nc.any.tensor_scalar_mul(out=oacc[i][:], in0=pv[:],
                                             scalar1=tot[:])
                else:
                    of = up.tile([P, D], f32, tag="of", name="of")
                    nc.any.tensor_scalar_mul(out=of[:], in0=pv[:], scalar1=tot[:])
                    of2 = up.tile([P, D], f32, tag="of2", name="of2")
                    nc.any.tensor_add(out=of2[:], in0=of[:], in1=oacc[i][:])
                    if final:
                        nc.sync.dma_start(out=out_ext[i * P:i * P + 64, :],
                                          in_=of2[0:64, :])
                        nc.scalar.dma_start(out=out_ext[i * P + 64:(i + 1) * P, :],
                                            in_=of2[64:128, :])
                    else:
                        nc.sync.dma_start(out=out_ext[i * P:(i + 1) * P, :], in_=of2[:])

            # ---- phase A: LayerNorm interleaved with both heads' qm ----
            for g in range(4):
                for i in range(4 * g, 4 * g + 4):
                    emit_ln_tile(i)
                emit_qm(0, at0, g)
                emit_qm(1, at1, g)

            # ---- PE filler queue: vm tiles for both heads ----
            emit_vm(0, nt0, 0)
            emit_vm(0, nt0, 1)
            filler = [(0, nt0, t) for t in range(2, NS)] + \
                     [(1, nt1, t) for t in range(NS)]
            fpos = 0

            # ---- attention rows: head 0 ascending, head 1 descending so the
            #      kernel drains on the cheapest row ----
            pend = None
            for h, i in [(0, i) for i in range(NS)] + [(1, i) for i in range(NS - 1, -1, -1)]:
                cur = (h, i) + emit_scores(h, i)
                npop = 3 if (h == 0 and i < 6) else 2
                for _ in range(npop):
                    if fpos < len(filler):
                        emit_vm(*filler[fpos])
                        fpos += 1
                if pend is not None:
                    emit_tail(*pend)
                pend = cur
            emit_tail(*pend, final=True)
    return nc


_NC = None


def _get_nc():
    global _NC
    if _NC is None:
        _NC = _build()
    return _NC


def _run(inputs, trace=False):
    x = np.asarray(inputs["x"], dtype=np.float32)          # [4, 2048, 512]
    gamma = np.asarray(inputs["gamma"], dtype=np.float32).reshape(D)
    beta = np.asarray(inputs["beta"], dtype=np.float32).reshape(D)
    Wq = np.asarray(inputs["Wq"], dtype=np.float32)        # [4, 512, 1024]
    Wk = np.asarray(inputs["Wk"], dtype=np.float32)
    Wv = np.asarray(inputs["Wv"], dtype=np.float32)
    Wout = np.asarray(inputs["Wout"], dtype=np.float32)    # [4096, 512]

    # Rank-D refactor: per head fold the QK^T and V-proj/out-proj pairs into
    # D x D matrices (U = 2D > D, so this more than halves the matmul work):
    #   scores = (z A + u) z^T      A = (g*Wq)(g*Wk)^T,  u = (b Wq)(g*Wk)^T
    #   head @ Wout = probs (z N) + (b Wv) Wout   N = (g*Wv) Wout
    # LN beta terms on the query side cancel in softmax; (b Wv) Wout is a
    # constant vector added host-side.
    H = 4
    A = np.empty((H, D, D), np.float32)
    Nm = np.empty((H, D, D), np.float32)
    ubias = np.empty((H, D), np.float32)
    cvec = np.zeros(D, np.float32)
    for h in range(H):
        Wkg = Wk[h] * gamma[:, None]
        A[h] = (Wq[h] * gamma[:, None]) @ Wkg.T
        ubias[h] = (beta @ Wq[h]) @ Wkg.T
        Nm[h] = (Wv[h] * gamma[:, None]) @ Wout[h * U:(h + 1) * U]
        cvec += (beta @ Wv[h]) @ Wout[h * U:(h + 1) * U]

    in_maps = []
    for c in range(8):
        b, hp = c // 2, c % 2
        ub = ubias[2 * hp:2 * hp + 2].reshape(2, ND, P).transpose(2, 0, 1).reshape(P, 2 * ND)
        in_maps.append({
            "x": np.ascontiguousarray(x[b]),
            "ub": np.ascontiguousarray(ub),
            "a": np.ascontiguousarray(A[2 * hp:2 * hp + 2].reshape(2 * D, D)).astype(np.float16),
            "n": np.ascontiguousarray(Nm[2 * hp:2 * hp + 2].reshape(2 * D, D)).astype(np.float16),
        })
    res = run_bass_kernel_spmd(_get_nc(), in_maps, list(range(8)), trace=trace)
    out = np.empty((4, S, D), np.float32)
    for b in range(4):
        out[b] = res.results[2 * b]["out"] + res.results[2 * b + 1]["out"] + cvec[None, :]
    return out, res


def kernel(**inputs):
    out, _ = _run(inputs, trace=False)
    return out
